# revision 17
# baseline (speedup 1.0000x reference)
"""Trainium2 Bass kernel for the correlation-map embedding module.

Math (per (b, nf) pair):
  f1d = bilinear_down28(feature_i[b, nf])                  # [C, 28, 28]
  f2sel[c, k] = bilinear sample of feature_j[b, nf] at the K knn grid points
  corr[k, :, :] = relu(sum_c f2sel[c, k] * f1d[c, :, :])   # [K, 28, 28]
  out[k] = corr[k] / sum_hw(exp(corr[k])) * 10

Key restructurings vs the reference:
  - only the K=128 selected query positions of f2 are ever computed (4-tap
    gather on GPSIMD, tap weighting + tap reduction on DVE), so every matmul
    shares one stationary f2sel operand;
  - the f1 bilinear downsample never materializes: each input element of the
    56x56 plane contributes to exactly one 28x28 output cell with one product
    weight, so a single contiguous full-plane multiply (f1 * W4full, one DVE
    op per half) replaces the strided 4-tap mul/add tree, and the 2x2-block
    reduction folds into 4 accumulating matmuls whose rhs are the even/odd
    strided views of the weighted plane;
  - the channel contraction runs on the tensor engine in float32r;
  - epilogue on ScalarE: relu(corr)*10 via activation scale, exp via Exp with
    scale=0.1 + accum_out, normalize via DVE tensor_scalar;
  - per nf, f2 loads before f1 so the gather -> weight -> reduce chain runs
    while f1 is still streaming (the gather chain is the longest dependency).

Sharding: pure data parallel - batch dim (16) split across 8 cores, 2 each.
"""

import numpy as np

# hardcoded problem shapes (grading calls kernel(**inputs) standalone)
B, NF, C, H, W = 16, 3, 128, 56, 56
G = 28
K = 128
NCORES = 8
BPC = B // NCORES  # 2
P = 128
HH = H // 2  # 28 input rows per half
GH = G // 2  # 14 output rows per half

_CACHE = {}


def _axis_coords(n_in):
    # float32 arithmetic to match the jax reference bit-for-bit
    src = np.arange(G, dtype=np.float32) * np.float32((n_in - 1) / (G - 1))
    i0 = np.clip(np.floor(src).astype(np.int32), 0, n_in - 2)
    w = (src - i0.astype(np.float32)).astype(np.float32)
    return i0, w


def _host_consts(knn_inds):
    i0h, wh = _axis_coords(H)
    i0w, ww = _axis_coords(W)
    # the even/odd strided-AP downsample assumes taps are (2k, 2k+1)
    assert np.array_equal(i0h, 2 * np.arange(G)) and np.array_equal(i0w, 2 * np.arange(G))

    ah, bh = (1.0 - wh), wh
    aw, bw = (1.0 - ww), ww
    # full-plane product weights: input element (2h'+u, 2w'+t) belongs to
    # output cell (h', w') with weight wh_tap[u][h'] * ww_tap[t][w']
    whfull = np.empty(H, dtype=np.float32)
    whfull[0::2] = ah
    whfull[1::2] = bh
    wwfull = np.empty(W, dtype=np.float32)
    wwfull[0::2] = aw
    wwfull[1::2] = bw
    w4full = np.outer(whfull, wwfull).astype(np.float32).reshape(-1)  # [3136]

    # gather indices/weights for the 4 bilinear taps of each knn point.
    # the f2 plane is pre-interleaved on-chip into rpp[c, p2, 4] (the 2x2
    # patch of raw-f2 taps for downsampled cell p2 stored contiguously), so
    # one d=4 gather index per knn point fetches its whole patch - the
    # GPSIMD ap_gather cost is per-index, so this halves it
    knn = np.asarray(knn_inds).astype(np.int64)  # [NF, K, 2]
    gidx = np.zeros((NF, P, K // 16), dtype=np.int16)
    gwts = np.zeros((NF, 4 * K), dtype=np.float32)
    for nf in range(NF):
        h2 = knn[nf, :, 1]
        w2 = knn[nf, :, 0]
        pos = (h2 * G + w2).astype(np.int64)  # [K] patch index, units of d=4
        wt = np.stack(
            [ah[h2] * aw[w2], ah[h2] * bw[w2], bh[h2] * aw[w2], bh[h2] * bw[w2]],
            axis=1,
        ).reshape(-1)
        gwts[nf] = wt.astype(np.float32)
        # ap_gather index layout: gathered index j comes from partition j%16,
        # slot j//16 of its 16-partition group; replicate across the 8 groups
        wrapped = pos.reshape(K // 16, 16).T.astype(np.int16)  # [16, 8]
        gidx[nf] = np.tile(wrapped, (8, 1))
    return w4full, gidx, gwts


def _build_bass():
    import concourse.bacc as bacc
    import concourse.tile as tile
    from concourse import mybir

    f32 = mybir.dt.float32
    f32r = mybir.dt.float32r
    bf16 = mybir.dt.bfloat16
    i16 = mybir.dt.int16
    AF = mybir.ActivationFunctionType
    ALU = mybir.AluOpType

    nc = bacc.Bacc()
    # fi declared f32r so the in-place weighted plane feeds the PE at full rate
    fi = nc.dram_tensor("fi", [BPC, NF, C, H, W], f32r, kind="ExternalInput")
    fj = nc.dram_tensor("fj", [BPC, NF, C, H, W], f32, kind="ExternalInput")
    w4_d = nc.dram_tensor("w4", [1, H * W + P], f32r, kind="ExternalInput")
    gidx_d = nc.dram_tensor("gidx", [NF, P, K // 16], i16, kind="ExternalInput")
    gw_d = nc.dram_tensor("gw", [1, NF * 4 * K], f32r, kind="ExternalInput")
    out_d = nc.dram_tensor("out", [BPC, NF, K, G, G], f32, kind="ExternalOutput")

    with tile.TileContext(nc) as tc:
        with (
            tc.tile_pool(name="consts", bufs=1) as consts,
            tc.tile_pool(name="feat2", bufs=2) as feat2,
            tc.tile_pool(name="feat1", bufs=2) as feat1,
            tc.tile_pool(name="work", bufs=2) as work,
            tc.tile_pool(name="gpool", bufs=3) as gpool,
            tc.tile_pool(name="psum", bufs=3, space="PSUM") as pspool,
            tc.tile_pool(name="bcpsum", bufs=2, space="PSUM") as bcpool,
            tc.tile_pool(name="outp", bufs=2) as outp,
            tc.tile_pool(name="opool", bufs=3) as opool,
        ):
            # dummy warmup gather on memset inputs (no DMA deps): forces the
            # GPSIMD ext-isa library load (MODIFY_POOL_CONFIG + ~6us IRAM
            # fetch) to happen during the initial DMA fill instead of
            # serializing the first real gather
            zsrc = consts.tile([P, 32], f32, tag="zsrc")
            nc.vector.memset(zsrc, 0.0)
            zidx = consts.tile([P, 1], i16, tag="zidx")
            nc.vector.memset(zidx, 0)
            gdummy = consts.tile([P, 32], f32, tag="gdummy")
            nc.gpsimd.ap_gather(
                gdummy, zsrc, zidx, channels=P, num_elems=16, d=2, num_idxs=16
            )

            # constants: tiny single-partition rows, loaded on the sync ring
            # BEFORE the feature loads so they complete in the ramp-up window
            # instead of queueing behind MBs of feature traffic. Then
            # replicate across partitions with ones-vector matmuls on the
            # idle PE + copies on the idle DVE. float32r rounding of the
            # weights (~1e-3) is in the same class as the matmul's own.
            w4row = consts.tile([1, H * W + P], f32r, tag="w4row")
            nc.sync.dma_start(out=w4row, in_=w4_d[:, :])
            gwrow = consts.tile([1, NF * 4 * K], f32r, tag="gwrow")
            nc.sync.dma_start(out=gwrow, in_=gw_d[:, :])
            gidx_t = []
            for nf in range(NF):
                it = consts.tile([P, K // 16], i16, tag=f"gidx{nf}")
                nc.sync.dma_start(out=it, in_=gidx_d[nf])
                gidx_t.append(it)
            # trailing P entries of the w4 input are 1.0: the ones row for
            # the PE partition-broadcast matmuls
            ones = w4row[:, H * W : H * W + P]

            bc_count = [0]

            def pe_broadcast(row_ap, n):
                """[1, n] -> [P, n] via PE: out = ones.T @ row."""
                dst = consts.tile([P, n], f32, tag=f"bc{bc_count[0]}")
                bc_count[0] += 1
                done = 0
                while done < n:
                    chunk = min(512, n - done)
                    bps = bcpool.tile([P, 512], f32, tag="bps")
                    nc.tensor.matmul(
                        bps[:, :chunk],
                        lhsT=ones,
                        rhs=row_ap[:, done : done + chunk],
                        start=True,
                        stop=True,
                    )
                    nc.scalar.copy(dst[:, done : done + chunk], bps[:, :chunk])
                    done += chunk
                return dst

            w4bc = pe_broadcast(w4row[:, : H * W], H * W)  # [P, 3136]
            gw_t = [
                pe_broadcast(gwrow[:, nf * 4 * K : (nf + 1) * 4 * K], 4 * K)
                for nf in range(NF)
            ]

            # prepass: per nf emit [f2x DMAs -> interleave -> gathers] with
            # no epilogue work in between, so the in-order ScalarE/Pool
            # queues never serialize an interleave or gather behind an
            # earlier nf's epilogue. DMA order front-loads f2x: the gather
            # chain for nf is ready well before its f1 arrives.
            def emit_f2_chain(nf):
                f2x = feat2.tile([P, BPC, H * W], f32, tag="f2x")
                for b in range(BPC):
                    nc.sync.dma_start(
                        out=f2x[:, b, :],
                        in_=fj[b, nf].rearrange("p h w -> p (h w)"),
                    )
                # pre-interleave f2 into patch-contiguous bf16 layout
                # rpp[c, b, p2, (u,t)] so one d=4 index gathers a whole 2x2
                # patch (halves the per-index-priced ap_gather)
                rpp = gpool.tile([P, BPC, G * G, 4], bf16, tag="rpp")
                for b in range(BPC):
                    sv = f2x[:, b].rearrange(
                        "p (h uu w tt) -> p h uu w tt", h=G, uu=2, tt=2
                    )
                    dv = rpp[:, b].rearrange("p (h w) (uu tt) -> p h w uu tt", h=G, uu=2)
                    nc.scalar.copy(dv[:, :, :, 0, :], sv[:, :, 0, :, :])
                    nc.scalar.copy(dv[:, :, :, 1, :], sv[:, :, 1, :, :])
                g2 = {}
                for b in range(BPC):
                    g = gpool.tile([P, K, 4], bf16, tag=f"g{b}")
                    nc.gpsimd.ap_gather(
                        g.rearrange("p k t -> p (k t)"),
                        rpp[:, b].rearrange("p q t -> p (q t)"),
                        gidx_t[nf],
                        channels=P,
                        num_elems=G * G,
                        d=4,
                        num_idxs=K,
                    )
                    g2[b] = g
                return g2

            def emit_f1_loads(nf):
                f1h = {}
                for b in range(BPC):
                    f1h[b] = []
                    for h in range(2):
                        t = feat1.tile([P, HH, W], f32r, tag=f"f1_{b}_{h}")
                        nc.sync.dma_start(
                            out=t, in_=fi[b, nf, :, h * HH : (h + 1) * HH, :]
                        )
                        f1h[b].append(t)
                return f1h

            def compute(nf, g2, f1h):
                # fused tap weighting: one contiguous shrinking-cast mul into
                # the same tile (bf16 write trails the f32 read), then the
                # row-pair (u) tap fold on DVE in bf16 2x mode into the
                # tile's tail region; only the column (t) fold stays on the
                # PE, so each (b, half) needs 2 bf16 matmuls instead of 4
                wtv = {}
                for b in range(BPC):
                    wtv[b] = []
                    for h in range(2):
                        # [P, 2*HH*W] bf16 view of the same tile bytes
                        wb = f1h[b][h].bitcast(bf16).rearrange("p h w -> p (h w)")
                        nc.vector.tensor_mul(
                            wb[:, : HH * W],
                            f1h[b][h].rearrange("p h w -> p (h w)"),
                            w4bc[:, h * HH * W : (h + 1) * HH * W],
                        )
                        usum = wb[:, HH * W : HH * W + GH * W].rearrange(
                            "p (h w) -> p h w", w=W
                        )  # [P, 14, 56] scratch in the tile tail
                        wt2 = wb[:, : HH * W].rearrange(
                            "p (h uu w) -> p h uu w", uu=2, w=W
                        )
                        nc.vector.tensor_add(usum, wt2[:, :, 0, :], wt2[:, :, 1, :])
                        wtv[b].append(usum)

                f2sel = {}
                for b in range(BPC):
                    gg = work.tile([P, K, 4], f32r, tag="gg")
                    nc.vector.tensor_mul(
                        gg.rearrange("p k t -> p (k t)"),
                        g2[b].rearrange("p k t -> p (k t)"),
                        gw_t[nf],
                    )
                    fs = work.tile([P, K], bf16, tag=f"fs{b}")
                    with nc.allow_low_precision(reason="bf16 products, fp32 psum"):
                        nc.vector.tensor_reduce(
                            fs, gg, axis=mybir.AxisListType.X, op=ALU.add
                        )
                    f2sel[b] = fs

                for b in range(BPC):
                    ps = pspool.tile([P, 2, 512], f32, tag="ps")
                    for h in range(2):
                        f1v = wtv[b][h].rearrange("p h (w tt) -> p h w tt", tt=2)
                        for t in range(2):
                            nc.tensor.matmul(
                                ps[:, h, : GH * G],
                                lhsT=f2sel[b],
                                rhs=f1v[:, :, :, t],
                                start=(t == 0),
                                stop=(t == 1),
                            )

                    # epilogue on ScalarE: r = 10*relu(corr);
                    # s = sum(exp(r/10)); out = r * (1/s)
                    r = outp.tile([P, 2, GH * G], f32, tag="r")
                    nc.scalar.activation(r, ps[:, :, : GH * G], AF.Relu, scale=10.0)
                    rf = r.rearrange("p h q -> p (h q)")  # [P, 784] contiguous
                    e = outp.tile([P, G * G], f32, tag="e")
                    s = work.tile([P, 1], f32, tag="s")
                    nc.scalar.activation(e, rf, AF.Exp, scale=0.1, accum_out=s)
                    rec = work.tile([P, 1], f32, tag="rec")
                    nc.vector.reciprocal(rec, s)
                    o = opool.tile([P, G * G], f32, tag="o")
                    # final normalize on ScalarE: Copy with per-partition scale
                    nc.scalar.activation(o, rf, AF.Copy, scale=rec)
                    # issue the store from ScalarE: keeps the SP/sync stream
                    # free to prefetch later pairs
                    nc.scalar.dma_start(
                        out=out_d[b, nf].rearrange("k g1 g2 -> k (g1 g2)"), in_=o
                    )

            # emission order keeps every in-order engine queue free of
            # late-blocking work ahead of ready work: nf2's interleave lands
            # on ScalarE between epilogue groups, and the sync-ring DMA
            # order front-loads f2x so each gather chain starts early
            g2s, f1hs = {}, {}
            g2s[0] = emit_f2_chain(0)
            g2s[1] = emit_f2_chain(1)
            f1hs[0] = emit_f1_loads(0)
            compute(0, g2s[0], f1hs[0])
            g2s[2] = emit_f2_chain(2)
            f1hs[1] = emit_f1_loads(1)
            compute(1, g2s[1], f1hs[1])
            f1hs[2] = emit_f1_loads(2)
            compute(2, g2s[2], f1hs[2])
    return nc


def _get_bass():
    if "nc" not in _CACHE:
        nc = _build_bass()
        # run the Bacc passes (reg alloc, library-load insertion) before the
        # PJRT path serializes the module
        if not nc.is_finalized():
            nc.finalize()
        _CACHE["nc"] = nc
    return _CACHE["nc"]


def kernel(feature_i, feature_j, mask, optical_flow, knn_inds):
    from concourse import bass_utils

    nc = _get_bass()
    w4full, gidx, gwts = _host_consts(knn_inds)

    fi = np.ascontiguousarray(np.asarray(feature_i, dtype=np.float32))
    fj = np.ascontiguousarray(np.asarray(feature_j, dtype=np.float32))
    w4in = np.concatenate([w4full, np.ones(P, np.float32)])[None, :]

    in_maps = []
    for core in range(NCORES):
        lo = core * BPC
        in_maps.append(
            {
                "fi": fi[lo : lo + BPC],
                "fj": fj[lo : lo + BPC],
                "w4": w4in,
                "gidx": gidx,
                "gw": gwts.reshape(1, -1),
            }
        )

    res = bass_utils.run_bass_kernel_spmd(nc, in_maps, core_ids=list(range(NCORES)))
    out = np.concatenate([res.results[c]["out"] for c in range(NCORES)], axis=0)
    return out.astype(np.float32)


# revision 19
# speedup vs baseline: 1.2130x; 1.2130x over previous
"""Trainium2 Bass kernel for the correlation-map embedding module.

Math (per (b, nf) pair):
  f1d = bilinear_down28(feature_i[b, nf])                  # [C, 28, 28]
  f2sel[c, k] = bilinear sample of feature_j[b, nf] at the K knn grid points
  corr[k, :, :] = relu(sum_c f2sel[c, k] * f1d[c, :, :])   # [K, 28, 28]
  out[k] = corr[k] / sum_hw(exp(corr[k])) * 10

Key restructurings vs the reference:
  - only the K=128 selected query positions of f2 are ever computed: f2 is
    pre-interleaved on ScalarE into a patch-contiguous bf16 layout
    rpp[c, p2, 4] so ONE d=4 ap_gather index per knn point fetches its whole
    2x2 tap patch (the GPSIMD gather prices per index; this halves it), then
    tap weighting + tap reduction on DVE give one stationary f2sel per pair;
  - the f1 bilinear downsample never materializes: each input element of the
    56x56 plane contributes to exactly one 28x28 output cell with one product
    weight, so a single contiguous in-place multiply (f1 * W4full, one DVE op
    per half) replaces the strided 4-tap mul/add tree, and the 2x2-block
    reduction folds into 4 accumulating matmuls whose rhs are the even/odd
    strided views of the weighted plane;
  - the channel contraction runs on the tensor engine in float32r;
  - epilogue on ScalarE: relu(corr)*10 via activation scale, exp via Exp with
    scale=0.1 + accum_out, normalize via Copy with per-partition scale;
  - scheduling: a warmup gather hoists the GPSIMD library load into the ramp
    window; the DMA order front-loads f2x (consts, f2x0, f2x1, f1_0, f2x2,
    f1_1, f1_2) and per-nf [f2x -> interleave -> gather] chains are emitted
    with no epilogue work between them, so no in-order engine queue ever
    holds a late-blocking wait ahead of ready work (Tile hoists wait
    conditions to earlier instructions on the same engine).

Sharding: pure data parallel - batch dim (16) split across 8 cores, 2 each.
"""

import numpy as np

# hardcoded problem shapes (grading calls kernel(**inputs) standalone)
B, NF, C, H, W = 16, 3, 128, 56, 56
G = 28
K = 128
NCORES = 8
BPC = B // NCORES  # 2
P = 128
HH = H // 2  # 28 input rows per half
GH = G // 2  # 14 output rows per half

_CACHE = {}


def _axis_coords(n_in):
    # float32 arithmetic to match the jax reference bit-for-bit
    src = np.arange(G, dtype=np.float32) * np.float32((n_in - 1) / (G - 1))
    i0 = np.clip(np.floor(src).astype(np.int32), 0, n_in - 2)
    w = (src - i0.astype(np.float32)).astype(np.float32)
    return i0, w


def _host_consts(knn_inds):
    i0h, wh = _axis_coords(H)
    i0w, ww = _axis_coords(W)
    # the even/odd strided-AP downsample assumes taps are (2k, 2k+1)
    assert np.array_equal(i0h, 2 * np.arange(G)) and np.array_equal(i0w, 2 * np.arange(G))

    ah, bh = (1.0 - wh), wh
    aw, bw = (1.0 - ww), ww
    # full-plane product weights: input element (2h'+u, 2w'+t) belongs to
    # output cell (h', w') with weight wh_tap[u][h'] * ww_tap[t][w']
    whfull = np.empty(H, dtype=np.float32)
    whfull[0::2] = ah
    whfull[1::2] = bh
    wwfull = np.empty(W, dtype=np.float32)
    wwfull[0::2] = aw
    wwfull[1::2] = bw
    w4full = np.outer(whfull, wwfull).astype(np.float32).reshape(-1)  # [3136]

    # gather indices/weights for the 4 bilinear taps of each knn point.
    # the f2 plane is pre-interleaved on-chip into rpp[c, p2, 4] (the 2x2
    # patch of raw-f2 taps for downsampled cell p2 stored contiguously), so
    # one d=4 gather index per knn point fetches its whole patch - the
    # GPSIMD ap_gather cost is per-index, so this halves it
    knn = np.asarray(knn_inds).astype(np.int64)  # [NF, K, 2]
    gidx = np.zeros((NF, P, K // 16), dtype=np.int16)
    gwts = np.zeros((NF, 4 * K), dtype=np.float32)
    for nf in range(NF):
        h2 = knn[nf, :, 1]
        w2 = knn[nf, :, 0]
        pos = (h2 * G + w2).astype(np.int64)  # [K] patch index, units of d=4
        wt = np.stack(
            [ah[h2] * aw[w2], ah[h2] * bw[w2], bh[h2] * aw[w2], bh[h2] * bw[w2]],
            axis=1,
        ).reshape(-1)
        gwts[nf] = wt.astype(np.float32)
        # ap_gather index layout: gathered index j comes from partition j%16,
        # slot j//16 of its 16-partition group; replicate across the 8 groups
        wrapped = pos.reshape(K // 16, 16).T.astype(np.int16)  # [16, 8]
        gidx[nf] = np.tile(wrapped, (8, 1))
    return w4full, gidx, gwts


def _build_bass():
    import concourse.bacc as bacc
    import concourse.tile as tile
    from concourse import mybir

    f32 = mybir.dt.float32
    f32r = mybir.dt.float32r
    bf16 = mybir.dt.bfloat16
    i16 = mybir.dt.int16
    AF = mybir.ActivationFunctionType
    ALU = mybir.AluOpType

    nc = bacc.Bacc()
    # fi declared f32r: the in-place weighted plane feeds the PE at full rate
    fi = nc.dram_tensor("fi", [BPC, NF, C, H, W], f32r, kind="ExternalInput")
    fj = nc.dram_tensor("fj", [BPC, NF, C, H, W], f32, kind="ExternalInput")
    w4_d = nc.dram_tensor("w4", [1, H * W + P], f32r, kind="ExternalInput")
    gidx_d = nc.dram_tensor("gidx", [NF, P, K // 16], i16, kind="ExternalInput")
    gw_d = nc.dram_tensor("gw", [1, NF * 4 * K], f32r, kind="ExternalInput")
    out_d = nc.dram_tensor("out", [BPC, NF, K, G, G], f32, kind="ExternalOutput")

    with tile.TileContext(nc) as tc:
        with (
            tc.tile_pool(name="consts", bufs=1) as consts,
            tc.tile_pool(name="feat2", bufs=2) as feat2,
            tc.tile_pool(name="feat1", bufs=2) as feat1,
            tc.tile_pool(name="work", bufs=2) as work,
            tc.tile_pool(name="gpool", bufs=3) as gpool,
            tc.tile_pool(name="psum", bufs=3, space="PSUM") as pspool,
            tc.tile_pool(name="bcpsum", bufs=2, space="PSUM") as bcpool,
            tc.tile_pool(name="outp", bufs=2) as outp,
            tc.tile_pool(name="opool", bufs=3) as opool,
        ):
            # dummy warmup gather on memset inputs (no DMA deps): forces the
            # GPSIMD ext-isa library load (MODIFY_POOL_CONFIG + ~6us IRAM
            # fetch) to happen during the initial DMA fill instead of
            # serializing the first real gather
            zsrc = consts.tile([P, 32], f32, tag="zsrc")
            nc.vector.memset(zsrc, 0.0)
            zidx = consts.tile([P, 1], i16, tag="zidx")
            nc.vector.memset(zidx, 0)
            gdummy = consts.tile([P, 32], f32, tag="gdummy")
            nc.gpsimd.ap_gather(
                gdummy, zsrc, zidx, channels=P, num_elems=16, d=2, num_idxs=16
            )

            # constants: tiny single-partition rows, loaded on the sync ring
            # BEFORE the feature loads so they complete in the ramp-up window
            # instead of queueing behind MBs of feature traffic. Then
            # replicate across partitions with ones-vector matmuls on the
            # idle PE + copies on the idle DVE. float32r rounding of the
            # weights (~1e-3) is in the same class as the matmul's own.
            w4row = consts.tile([1, H * W + P], f32r, tag="w4row")
            nc.sync.dma_start(out=w4row, in_=w4_d[:, :])
            gwrow = consts.tile([1, NF * 4 * K], f32r, tag="gwrow")
            nc.sync.dma_start(out=gwrow, in_=gw_d[:, :])
            gidx_t = []
            for nf in range(NF):
                it = consts.tile([P, K // 16], i16, tag=f"gidx{nf}")
                nc.sync.dma_start(out=it, in_=gidx_d[nf])
                gidx_t.append(it)
            # trailing P entries of the w4 input are 1.0: the ones row for
            # the PE partition-broadcast matmuls
            ones = w4row[:, H * W : H * W + P]

            bc_count = [0]

            def pe_broadcast(row_ap, n):
                """[1, n] -> [P, n] via PE: out = ones.T @ row."""
                dst = consts.tile([P, n], f32, tag=f"bc{bc_count[0]}")
                bc_count[0] += 1
                done = 0
                while done < n:
                    chunk = min(512, n - done)
                    bps = bcpool.tile([P, 512], f32, tag="bps")
                    nc.tensor.matmul(
                        bps[:, :chunk],
                        lhsT=ones,
                        rhs=row_ap[:, done : done + chunk],
                        start=True,
                        stop=True,
                    )
                    nc.scalar.copy(dst[:, done : done + chunk], bps[:, :chunk])
                    done += chunk
                return dst

            w4bc = pe_broadcast(w4row[:, : H * W], H * W)  # [P, 3136]
            gw_t = [
                pe_broadcast(gwrow[:, nf * 4 * K : (nf + 1) * 4 * K], 4 * K)
                for nf in range(NF)
            ]

            # prepass: per nf emit [f2x DMAs -> interleave -> gathers] with
            # no epilogue work in between, so the in-order ScalarE/Pool
            # queues never serialize an interleave or gather behind an
            # earlier nf's epilogue. DMA order front-loads f2x: the gather
            # chain for nf is ready well before its f1 arrives.
            def emit_f2_chain(nf):
                f2x = feat2.tile([P, BPC, H * W], f32, tag="f2x")
                for b in range(BPC):
                    nc.sync.dma_start(
                        out=f2x[:, b, :],
                        in_=fj[b, nf].rearrange("p h w -> p (h w)"),
                    )
                # pre-interleave f2 into patch-contiguous bf16 layout
                # rpp[c, b, p2, (u,t)] so one d=4 index gathers a whole 2x2
                # patch (halves the per-index-priced ap_gather)
                rpp = gpool.tile([P, BPC, G * G, 4], bf16, tag="rpp")
                for b in range(BPC):
                    sv = f2x[:, b].rearrange(
                        "p (h uu w tt) -> p h uu w tt", h=G, uu=2, tt=2
                    )
                    dv = rpp[:, b].rearrange("p (h w) (uu tt) -> p h w uu tt", h=G, uu=2)
                    nc.scalar.copy(dv[:, :, :, 0, :], sv[:, :, 0, :, :])
                    nc.scalar.copy(dv[:, :, :, 1, :], sv[:, :, 1, :, :])
                g2 = {}
                for b in range(BPC):
                    g = gpool.tile([P, K, 4], bf16, tag=f"g{b}")
                    nc.gpsimd.ap_gather(
                        g.rearrange("p k t -> p (k t)"),
                        rpp[:, b].rearrange("p q t -> p (q t)"),
                        gidx_t[nf],
                        channels=P,
                        num_elems=G * G,
                        d=4,
                        num_idxs=K,
                    )
                    g2[b] = g
                return g2

            def emit_f1_loads(nf):
                f1h = {}
                for b in range(BPC):
                    f1h[b] = []
                    for h in range(2):
                        t = feat1.tile([P, HH, W], f32r, tag=f"f1_{b}_{h}")
                        nc.sync.dma_start(
                            out=t, in_=fi[b, nf, :, h * HH : (h + 1) * HH, :]
                        )
                        f1h[b].append(t)
                return f1h

            g2s, f1hs = {}, {}
            g2s[0] = emit_f2_chain(0)
            g2s[1] = emit_f2_chain(1)
            f1hs[0] = emit_f1_loads(0)
            g2s[2] = emit_f2_chain(2)
            f1hs[1] = emit_f1_loads(1)
            f1hs[2] = emit_f1_loads(2)

            for nf in range(NF):
                g2 = g2s[nf]
                f1h = f1hs[nf]

                # fused tap weighting: one contiguous in-place mul replaces
                # the 4-tap strided mul/add tree; runs while gathers proceed
                for b in range(BPC):
                    for h in range(2):
                        fh = f1h[b][h].rearrange("p h w -> p (h w)")
                        nc.vector.tensor_mul(
                            fh, fh, w4bc[:, h * HH * W : (h + 1) * HH * W]
                        )

                f2sel = {}
                for b in range(BPC):
                    gg = work.tile([P, K, 4], f32r, tag="gg")
                    nc.vector.tensor_mul(
                        gg.rearrange("p k t -> p (k t)"),
                        g2[b].rearrange("p k t -> p (k t)"),
                        gw_t[nf],
                    )
                    fs = work.tile([P, K], f32r, tag=f"fs{b}")
                    with nc.allow_low_precision(reason="f32r is fp32-width"):
                        nc.vector.tensor_reduce(
                            fs, gg, axis=mybir.AxisListType.X, op=ALU.add
                        )
                    f2sel[b] = fs

                for b in range(BPC):
                    ps = pspool.tile([P, 2, 512], f32, tag="ps")
                    for h in range(2):
                        # 2x2-block reduction on the PE: 4 accumulating
                        # matmuls over the even/odd strided views of the
                        # weighted plane
                        f1v = f1h[b][h].rearrange(
                            "p (h uu) (w tt) -> p h uu w tt", uu=2, tt=2
                        )
                        i = 0
                        for u in range(2):
                            for t in range(2):
                                nc.tensor.matmul(
                                    ps[:, h, : GH * G],
                                    lhsT=f2sel[b],
                                    rhs=f1v[:, :, u, :, t],
                                    start=(i == 0),
                                    stop=(i == 3),
                                )
                                i += 1

                    # epilogue on ScalarE: r = 10*relu(corr);
                    # s = sum(exp(r/10)); out = r * (1/s)
                    r = outp.tile([P, 2, GH * G], f32, tag="r")
                    nc.scalar.activation(r, ps[:, :, : GH * G], AF.Relu, scale=10.0)
                    rf = r.rearrange("p h q -> p (h q)")  # [P, 784] contiguous
                    e = outp.tile([P, G * G], f32, tag="e")
                    s = work.tile([P, 1], f32, tag="s")
                    nc.scalar.activation(e, rf, AF.Exp, scale=0.1, accum_out=s)
                    rec = work.tile([P, 1], f32, tag="rec")
                    nc.vector.reciprocal(rec, s)
                    o = opool.tile([P, G * G], f32, tag="o")
                    # final normalize on ScalarE: Copy with per-partition scale
                    nc.scalar.activation(o, rf, AF.Copy, scale=rec)
                    # issue the store from ScalarE (mostly idle): keeps the
                    # SP/sync stream free to prefetch later pairs
                    nc.scalar.dma_start(
                        out=out_d[b, nf].rearrange("k g1 g2 -> k (g1 g2)"), in_=o
                    )
    return nc


def _get_bass():
    if "nc" not in _CACHE:
        nc = _build_bass()
        # run the Bacc passes (reg alloc, library-load insertion) before the
        # PJRT path serializes the module
        if not nc.is_finalized():
            nc.finalize()
        _CACHE["nc"] = nc
    return _CACHE["nc"]


def kernel(feature_i, feature_j, mask, optical_flow, knn_inds):
    from concourse import bass_utils

    nc = _get_bass()
    w4full, gidx, gwts = _host_consts(knn_inds)

    fi = np.ascontiguousarray(np.asarray(feature_i, dtype=np.float32))
    fj = np.ascontiguousarray(np.asarray(feature_j, dtype=np.float32))
    w4in = np.concatenate([w4full, np.ones(P, np.float32)])[None, :]

    in_maps = []
    for core in range(NCORES):
        lo = core * BPC
        in_maps.append(
            {
                "fi": fi[lo : lo + BPC],
                "fj": fj[lo : lo + BPC],
                "w4": w4in,
                "gidx": gidx,
                "gw": gwts.reshape(1, -1),
            }
        )

    res = bass_utils.run_bass_kernel_spmd(nc, in_maps, core_ids=list(range(NCORES)))
    out = np.concatenate([res.results[c]["out"] for c in range(NCORES)], axis=0)
    return out.astype(np.float32)
